# revision 29
# baseline (speedup 1.0000x reference)
import numpy as np
import concourse.bass as bass
import concourse.tile as tile
from concourse import mybir
from concourse.bass_utils import run_bass_kernel_spmd
from concourse.masks import make_identity

P = 128
S = 2048
D = 512
U = 1024
NS = S // P      # 16 s-tiles
ND = D // P      # 4 d-blocks
NEG = -60000.0
EPS = 1e-6


def _patched_drain_and_barrier(self, tick_clock, wait_clock):
    nc = self.nc
    probe = nc.sync.nop(nofuse=True, hint="drain_waits_probe")
    wait_clock.add_sem_waits(probe.ins, tile.ScopedClock({None: tick_clock.global_clock}))
    si = probe.ins.sync_info
    waits = list(si.on_wait) if si is not None else []
    assert self.sems is not None
    handles = {h.name: h for h in self.sems.allocated().values()}
    if len(waits) > 1:
        import bass_rust
        probe.ins.sync_info = bass_rust.SyncInfo(on_wait=waits[:1], on_update=[])
        for w in waits[1:]:
            h = handles.get(w.ant_name)
            assert h is not None, (w.ant_name, list(handles))
            nc.sync.wait_ge(h, w.wait_value)
    nc.sync.drain()
    nc.all_engine_barrier()
    popped = nc._tile_sem_poison_stack.pop()
    assert popped is self._sem_poison
    nc.clear_and_free_semaphores(list(self.sems.allocated().values()))
    nc.all_engine_barrier()


tile.TileContext._drain_and_barrier = _patched_drain_and_barrier

# The walrus backend in this toolchain rejects instructions carrying more
# than one semaphore wait ("Too many sync wait commands"). Split excess
# waits onto single-wait NoOp carriers on the same engine, which execute
# in order ahead of the real instruction.
_MAXW = 1
_orig_lower_ordered = tile.TileContext._lower_ordered_insts


def _patched_lower_ordered(self, ordered):
    nc = self.nc
    for insts in ordered.values():
        out = []
        for inst in insts:
            si = getattr(inst, "sync_info", None)
            eng = getattr(inst, "engine", None)
            if (si is not None and si.on_wait and len(si.on_wait) > _MAXW
                    and eng is not None
                    and not type(inst).__name__.startswith("BassTile")):
                waits = list(si.on_wait)
                for w in waits[:-_MAXW]:
                    out.append(mybir.InstNoOp(
                        name=nc.get_next_instruction_name(),
                        engine=eng,
                        ins=[],
                        outs=[],
                        bass_nofuse=True,
                        sync_info=mybir.SyncInfo(on_wait=[w], on_update=[]),
                    ))
                inst.sync_info = mybir.SyncInfo(
                    on_wait=waits[-_MAXW:], on_update=list(si.on_update))
            out.append(inst)
        insts[:] = out
    return _orig_lower_ordered(self, ordered)


tile.TileContext._lower_ordered_insts = _patched_lower_ordered

f32 = mybir.dt.float32
f16 = mybir.dt.float16
bf16 = mybir.dt.bfloat16
ACT = mybir.ActivationFunctionType


def _build():
    # Per-core problem: one batch b, two heads (h=0,1 local).
    # Host has folded the per-head weight pairs:
    #   M_h = Wq'_h @ Wk'_h^T  [D, D]  (scores = z M z^T + 1 r z^T)
    #   N_h = Wv'_h @ Wout_h   [D, D]  (out = sum_h softmax(..) (z N_h) + cvec)
    # Scores are produced directly transposed ([key t, query q]) so no probs
    # transpose is needed; softmax uses exp with no max subtraction (probs in
    # bf16, whose exponent range covers e^|s|max ~ e^44), the row sum Z comes
    # from an N=1 ones-matmul reusing the PV stationary operand, and 1/Z is
    # folded into the PV-output rescale.
    nc = bass.Bass()
    x_ext = nc.declare_dram_parameter("x", [S, D], f32, isOutput=False)
    m_ext = nc.declare_dram_parameter("m", [2 * D, D], f16, isOutput=False)
    n_ext = nc.declare_dram_parameter("n", [2 * D, D], f16, isOutput=False)
    rt_ext = nc.declare_dram_parameter("rt", [P, 8], f32, isOutput=False)
    out_ext = nc.declare_dram_parameter("out", [S, D], f16, isOutput=True)

    with tile.TileContext(nc) as tc:
        with tc.tile_pool(name="const", bufs=1) as cp, \
             tc.tile_pool(name="zt", bufs=1) as zp, \
             tc.tile_pool(name="wp", bufs=1) as wp, \
             tc.tile_pool(name="qt", bufs=1) as qp, \
             tc.tile_pool(name="vt", bufs=1) as vp, \
             tc.tile_pool(name="pt", bufs=1) as pp, \
             tc.tile_pool(name="xd", bufs=6) as xdp, \
             tc.tile_pool(name="ln", bufs=2) as lp, \
             tc.tile_pool(name="outp", bufs=3) as up:

            # PSUM pools are scoped: the prologue (projection) pools close
            # before the attention pools open, so the phases time-share banks
            mmp = trp = scp = pvp = zsp = None

            ident = cp.tile([P, P], f16, tag="ident")
            make_identity(nc, ident[:])
            eps = cp.tile([P, 1], f32, tag="eps")
            nc.vector.memset(eps[:], EPS)
            # prime the scalar-engine Sqrt activation table while idle so the
            # first LayerNorm tile doesn't eat the ACT_TABLE_LOAD latency
            warm = cp.tile([P, 1], f32, tag="warm")
            nc.scalar.activation(out=warm[:], in_=eps[:], func=ACT.Sqrt,
                                 bias=eps[:], scale=1.0, alpha=0.0)
            ones = cp.tile([P, 1], bf16, tag="ones")
            nc.vector.memset(ones[:], 1.0)
            rt = cp.tile([P, 8], f32, tag="rt")
            nc.scalar.dma_start(out=rt[:], in_=rt_ext[:, :])
            # maskT[t, q] = 0 where q >= t else NEG   (diag block, [t,q] orient)
            maskT = cp.tile([P, P], f32, tag="maskT")
            nc.gpsimd.memset(maskT[:], 0.0)
            nc.gpsimd.affine_select(
                out=maskT[:],
                in_=maskT[:],
                compare_op=mybir.AluOpType.is_ge,
                fill=NEG,
                base=0,
                pattern=[[1, P]],
                channel_multiplier=-1,
            )

            zT = [zp.tile([P, S], f16, tag=f"zt{j}", name=f"zt{j}") for j in range(ND)]
            mt = [[wp.tile([P, D], f16, tag=f"m{h}{j}", name=f"m{h}{j}")
                   for j in range(ND)] for h in range(2)]
            nt = [[wp.tile([P, D], f16, tag=f"n{h}{j}", name=f"n{h}{j}")
                   for j in range(ND)] for h in range(2)]
            QT = [[qp.tile([P, S], f16, tag=f"q{h}{j}", name=f"q{h}{j}")
                   for j in range(ND)] for h in range(2)]
            VT = [[vp.tile([P, D], bf16, tag=f"v{h}{t}", name=f"v{h}{t}")
                   for t in range(NS)] for h in range(2)]
            PT = [[pp.tile([P, S - t * P], bf16, tag=f"p{h}{t}", name=f"p{h}{t}")
                   for t in range(NS)] for h in range(2)]

            dmaq = [nc.sync, nc.scalar, nc.gpsimd]
            xq = {0: 2, 3: 2}
            for _t in (1, 4, 6, 8, 10, 12, 14):
                xq[_t] = 0
            for _t in (2, 5, 7, 9, 11, 13, 15):
                xq[_t] = 1

            def load_m():
                for h in range(2):
                    for j in range(ND):
                        dmaq[(h * ND + j) % 3].dma_start(
                            out=mt[h][j][:],
                            in_=m_ext[h * D + j * P: h * D + (j + 1) * P, :])

            def load_n():
                for h in range(2):
                    for j in range(ND):
                        dmaq[(h * ND + j + 2) % 3].dma_start(
                            out=nt[h][j][:],
                            in_=n_ext[h * D + j * P: h * D + (j + 1) * P, :])

            def emit_ln_tile(i):
                xt = xdp.tile([P, D], f32, tag="x", name="xt")
                if i < 4:
                    # split early tiles across all queues to cut arrival time
                    q = dmaq[(2 * i) % 3]
                    q2 = dmaq[(2 * i + 1) % 3]
                    q.dma_start(out=xt[0:64, :], in_=x_ext[i * P:i * P + 64, :])
                    q2.dma_start(out=xt[64:128, :], in_=x_ext[i * P + 64:(i + 1) * P, :])
                else:
                    dmaq[xq[i]].dma_start(out=xt[:], in_=x_ext[i * P:(i + 1) * P, :])
                stats = lp.tile([P, 6], f32, tag="bs", name="bs")
                nc.vector.bn_stats(out=stats[:], in_=xt[:])
                mv = lp.tile([P, 2], f32, tag="mv", name="mv")
                nc.vector.bn_aggr(out=mv[:], in_=stats[:])
                sd = lp.tile([P, 1], f32, tag="sd", name="sd")
                nc.scalar.activation(out=sd[:], in_=mv[:, 1:2],
                                     func=ACT.Sqrt,
                                     bias=eps[:], scale=1.0, alpha=0.0)
                nc.vector.reciprocal(out=sd[:], in_=sd[:])
                xh = lp.tile([P, D], f16, tag="xh", name="xh")
                nc.vector.tensor_scalar(out=xh[:], in0=xt[:],
                                        scalar1=mv[:, 0:1], scalar2=sd[:],
                                        op0=mybir.AluOpType.subtract,
                                        op1=mybir.AluOpType.mult)
                for j in range(ND):
                    tp = trp.tile([P, P], f16, tag="tr", name="tp")
                    nc.tensor.transpose(tp[:], xh[:, j * P:(j + 1) * P], ident[:])
                    if (i * ND + j) % 2 == 0:
                        nc.vector.tensor_copy(out=zT[j][:, i * P:(i + 1) * P],
                                              in_=tp[:])
                    else:
                        nc.scalar.activation(out=zT[j][:, i * P:(i + 1) * P],
                                             in_=tp[:], func=ACT.Copy)

            def emit_qproj(h, g):
                # QT[h] [d', s-chunk g] = sum_j mt[h][j][:, d'-block].T @ zT[j][:, chunk]
                for u4 in range(ND):
                    mm = mmp.tile([P, 512], f32, tag="mm", name="mm")
                    for j in range(ND):
                        nc.tensor.matmul(mm[:],
                                         mt[h][j][:, u4 * P:(u4 + 1) * P],
                                         zT[j][:, g * 512:(g + 1) * 512],
                                         start=(j == 0), stop=(j == ND - 1))
                    if h == 0:
                        # scalar engine is idle during the LN phase; the
                        # Copy activation computes in*1 + bias
                        nc.scalar.activation(
                            out=QT[h][u4][:, g * 512:(g + 1) * 512],
                            in_=mm[:], func=ACT.Identity,
                            bias=rt[:, h * 4 + u4:h * 4 + u4 + 1], scale=1.0)
                    else:
                        nc.vector.tensor_scalar_add(
                            out=QT[h][u4][:, g * 512:(g + 1) * 512],
                            in0=mm[:],
                            scalar1=rt[:, h * 4 + u4:h * 4 + u4 + 1])

            def emit_v(h, t):
                # VT[h][t] [t-block, d'] = sum_j zT[j][:, t-block].T @ nt[h][j]
                mm = mmp.tile([P, 512], f32, tag="mm", name="mm")
                for j in range(ND):
                    nc.tensor.matmul(mm[:],
                                     zT[j][:, t * P:(t + 1) * P],
                                     nt[h][j][:],
                                     start=(j == 0), stop=(j == ND - 1))
                nc.scalar.activation(out=VT[h][t][:], in_=mm[:], func=ACT.Copy)

            def emit_scores(h, t):
                # PT[h][t][tloc, q - t*P] = exp(scoresT) for q in [t*P, S)
                q0 = t * P
                w_all = S - q0
                nch = (w_all + 511) // 512
                for c in range(nch):
                    w = min(512, w_all - c * 512)
                    sc = scp.tile([P, 512], f32, tag="sc", name="sc")
                    for j in range(ND):
                        nc.tensor.matmul(sc[:, 0:w],
                                         zT[j][:, q0:q0 + P],
                                         QT[h][j][:, q0 + c * 512:q0 + c * 512 + w],
                                         start=(j == 0), stop=(j == ND - 1))
                    if c == 0:
                        nc.vector.tensor_add(out=sc[:, 0:P], in0=sc[:, 0:P],
                                             in1=maskT[:])
                    nc.scalar.activation(out=PT[h][t][:, c * 512:c * 512 + w],
                                         in_=sc[:, 0:w], func=ACT.Exp)

            def emit_pv(h, i, o0=None):
                # out tile i for head h: (1/Z) * sum_tb PT[h][tb]^T @ VT[h][tb]
                pv = pvp.tile([P, 512], f32, tag="pv", name="pv")
                zs = zsp.tile([P, 1], f32, tag="zs", name="zs")
                for tb in range(i + 1):
                    lt = PT[h][tb][:, (i - tb) * P:(i - tb + 1) * P]
                    nc.tensor.matmul(pv[:], lt, VT[h][tb][:],
                                     start=(tb == 0), stop=(tb == i))
                    nc.tensor.matmul(zs[:], lt, ones[:],
                                     start=(tb == 0), stop=(tb == i))
                rc = up.tile([P, 1], f32, tag=f"rc{h}", name="rc")
                nc.vector.reciprocal(out=rc[:], in_=zs[:])
                if h == 0:
                    o0 = up.tile([P, D], f32, tag="o0", name="o0")
                    nc.vector.tensor_scalar_mul(out=o0[:], in0=pv[:], scalar1=rc[:])
                    return o0
                o1 = up.tile([P, D], f32, tag="o1", name="o1")
                nc.vector.tensor_scalar_mul(out=o1[:], in0=pv[:], scalar1=rc[:])
                of = up.tile([P, D], f16, tag="of", name="of")
                nc.vector.tensor_add(out=of[:], in0=o1[:], in1=o0[:])
                if i == NS - 1:
                    nc.sync.dma_start(out=out_ext[i * P:i * P + 64, :],
                                      in_=of[0:64, :])
                    nc.scalar.dma_start(out=out_ext[i * P + 64:(i + 1) * P, :],
                                        in_=of[64:128, :])
                else:
                    dmaq[i % 3].dma_start(out=out_ext[i * P:(i + 1) * P, :],
                                          in_=of[:])
                return None

            # ---- prologue: LayerNorm + Q'/V' projections ----
            # DMA order: x tiles of group 0 first, then M (needed by the
            # first projection) and N (needed by the first V' tiles).
            with tc.tile_pool(name="mm", bufs=2, space="PSUM") as mmp, \
                 tc.tile_pool(name="tr", bufs=3, space="PSUM") as trp:
                for g in range(4):
                    for i in range(4 * g, 4 * g + 4):
                        emit_ln_tile(i)
                    if g == 0:
                        load_m()
                        load_n()
                    emit_qproj(0, g)
                    emit_qproj(1, g)
                    for t in range(4 * g, 4 * g + 4):
                        emit_v(0, t)
                        emit_v(1, t)

            # ---- attention, key-block-major; PV pipelined one step behind ----
            with tc.tile_pool(name="sc", bufs=3, space="PSUM") as scp, \
                 tc.tile_pool(name="pv", bufs=2, space="PSUM") as pvp, \
                 tc.tile_pool(name="zp", bufs=1, space="PSUM") as zsp:
                o0_pend = None
                for t in range(NS):
                    emit_scores(0, t)
                    if t > 0:
                        o0_pend = emit_pv(0, t - 1)
                    emit_scores(1, t)
                    if t > 0:
                        emit_pv(1, t - 1, o0_pend)
                o0_pend = emit_pv(0, NS - 1)
                emit_pv(1, NS - 1, o0_pend)
    return nc


_NC = None


def _get_nc():
    global _NC
    if _NC is None:
        _NC = _build()
    return _NC


def _run(inputs, trace=False):
    x = np.asarray(inputs["x"], dtype=np.float32)          # [4, 2048, 512]
    gamma = np.asarray(inputs["gamma"], dtype=np.float32).reshape(D)
    beta = np.asarray(inputs["beta"], dtype=np.float32).reshape(D)
    Wq = np.asarray(inputs["Wq"], dtype=np.float32)        # [4, 512, 1024]
    Wk = np.asarray(inputs["Wk"], dtype=np.float32)
    Wv = np.asarray(inputs["Wv"], dtype=np.float32)
    Wout = np.asarray(inputs["Wout"], dtype=np.float32)    # [4096, 512]

    # fold LN gamma into the projections, then fold the weight pairs:
    #   scores_h = z (Wq'_h Wk'_h^T) z^T + 1 (bq_h Wk'_h^T) z^T  (+row-const)
    #   out     = sum_h P_h z (Wv'_h Wout_h) + 1 (bv_h Wout_h)
    Wqf = Wq * gamma[None, :, None]
    Wkf = Wk * gamma[None, :, None]
    Wvf = Wv * gamma[None, :, None]
    bq_all = np.einsum("d,hdu->hu", beta, Wq)              # [4, 1024]
    bv_all = np.einsum("d,hdu->hu", beta, Wv)              # [4, 1024]
    cvec = np.zeros(D, np.float32)
    for h in range(4):
        cvec += bv_all[h] @ Wout[h * U:(h + 1) * U]
    Mf = np.stack([Wqf[h] @ Wkf[h].T for h in range(4)])           # [4, D, D]
    Nf = np.stack([Wvf[h] @ Wout[h * U:(h + 1) * U] for h in range(4)])
    rv = np.stack([bq_all[h] @ Wkf[h].T for h in range(4)])        # [4, D]

    in_maps = []
    for c in range(8):
        b, hp = c // 2, c % 2
        hs = (2 * hp, 2 * hp + 1)
        m = np.concatenate([Mf[hs[0]], Mf[hs[1]]], axis=0)
        n = np.concatenate([Nf[hs[0]], Nf[hs[1]]], axis=0)
        rt = np.stack([rv[hs[hl]][u4 * P:(u4 + 1) * P]
                       for hl in range(2) for u4 in range(4)], axis=1)
        in_maps.append({
            "x": np.ascontiguousarray(x[b]),
            "m": np.ascontiguousarray(m).astype(np.float16),
            "n": np.ascontiguousarray(n).astype(np.float16),
            "rt": np.ascontiguousarray(rt).astype(np.float32),
        })
    res = run_bass_kernel_spmd(_get_nc(), in_maps, list(range(8)), trace=trace)
    out = np.empty((4, S, D), np.float32)
    for b in range(4):
        out[b] = (res.results[2 * b]["out"].astype(np.float32)
                  + res.results[2 * b + 1]["out"].astype(np.float32)
                  + cvec[None, :])
    return out, res


def kernel(**inputs):
    out, _ = _run(inputs, trace=False)
    return out
